# revision 35
# baseline (speedup 1.0000x reference)
"""Trainium2 Bass kernel for dilated 5x7 conv (128->16ch) + 1x1 (16->16) + 1x1 (16->128).

Strategy (data-parallel, 1 image per core across 8 cores):
  reference: y = conv_dilated(x, w3, dil=(2,3), pad=(4,9)); y = w4@y; y = w5@y
  Host folds w45 = w5 @ w4  [128, 16].

  Per core, image x [128, 56, 56] zero-padded AND W-major transposed to
  xp [c=128, w'=74, r=64] (bf16), xp[c, w', r] = xpad[c, r, w'].

  Stage 1 (TensorE, contract kh): for each kh in 0..4, matmul with
      lhsT = w1[:, kh, :] [c=128, (kw,co)=112], rhs = xp[:, wchunk, 2kh:2kh+56],
      PSUM-accumulating over kh  ->  P[(kw,co), w', h] =
      sum_{kh,c} w3[co,c,kh,kw] * xpad[c, h+2kh, w'].
  Evacuate PSUM->SBUF p2s [112, 74, 56] bf16 (w-major => any w-window of all
  h is CONTIGUOUS in the free dim).
  Shift-align for w<WSPLIT via SBUF->SBUF DMA (contiguous 16-partition runs):
      p2a[(kw,co), w, h] = p2s[(kw,co), w+3kw, h].
  Stage 2a (w in [WSPLIT, 56), block-diag, no shift/DMA dependency):
      out[o, w, h] += sum_co w45[o,co] * p2s[(g,co), w+3g, h]   (7 K=16 matmuls)
  Stage 2b (w in [0, WSPLIT), single matmul per chunk, K=112):
      out[o, w, h] = sum_{(kw,co)} w45[o,co] * p2a[(kw,co), w, h].
  Evacuate (RR engines) to bf16, DMA out; host casts f32 + transposes (w,h)->(h,w).
"""

import os
import sys

import numpy as np

for _p in ("/opt/trn_rl_repo", "/root/.axon_site/_ro/trn_rl_repo"):
    if os.path.isdir(_p) and _p not in sys.path:
        sys.path.insert(0, _p)

import ml_dtypes  # noqa: E402

import concourse.bass as bass  # noqa: E402
import concourse.tile as tile  # noqa: E402
from concourse.tile_rust import add_dep_helper  # noqa: E402
from concourse import mybir  # noqa: E402
from concourse.bass_utils import run_bass_kernel_spmd  # noqa: E402

N, C, H, W = 8, 128, 56, 56
CO = 16
KH, KW = 5, 7
DH, DW = 2, 3
PH, PW = 4, 9
RP, WP = H + 2 * PH, W + 2 * PW  # 64 padded rows, 74 padded cols
M1 = KW * CO  # 112 = (kw, co)
WSPLIT = 40  # w < WSPLIT via shift-DMA + single matmul; w >= WSPLIT block-diag
# stage-1 w' chunks (PSUM bank: <=512 fp32/partition => <=9 w' of 56 h)
S1_CH = [(0, 9), (9, 9), (18, 9), (27, 9), (36, 9), (45, 9), (54, 9), (63, 9), (72, 2)]
# shift DMAs need p2s w' <= 3*6 + WSPLIT - 1 = 57 -> stage-1 chunks 0..6
# xp input DMA pieces: boundaries land on stage-1 chunks 0 and 3, which use
# FRESH ps1 buffers (bufs=4) -> each boundary matmul carries only the DMA
# wait, never DMA wait + PSUM-bank-WAR wait (matmul has ONE wait slot).
# Total HWDGE DMAs = wk1 + 2 xp + 3 shifts + 2 outs = 8 = #physical queues,
# so NO queue is reused and NO DMA ever carries a queue-WAW wait. First xp
# piece is tiny so stage-1 chunk 0 starts as early as possible.
XP_PC = [(0, 9), (9, 65)]
BD_CH = [(40, 8), (48, 8)]  # block-diag stage-2 w chunks
S2_CH = [(0, 8), (8, 8), (16, 8), (24, 8), (32, 8)]  # shifted stage-2 w chunks
# out DMA pieces (w ranges), in issue order; last computed piece is small
# wk free cols: w1 (5*112) + w2 (128) + 7 zero-padded block-diag w2 blocks
W2OFF = KH * M1  # 560
BDOFF = W2OFF + 128  # 688
WKC = BDOFF + KW * 128  # 1584
BF16 = mybir.dt.bfloat16
F32 = mybir.dt.float32

_NC = None


def _build_nc(attempt=0):
    nc = bass.Bass(
        "TRN2",
        target_bir_lowering=False,
        debug=False,
        enable_asserts=False,
        num_devices=N,
    )
    xp_d = nc.dram_tensor("xp", [C, WP, RP], BF16, kind="ExternalInput")
    wk_d = nc.dram_tensor("wk", [C, WKC], BF16, kind="ExternalInput")
    out_d = nc.dram_tensor("out", [C, W * H], BF16, kind="ExternalOutput")

    with tile.TileContext(nc) as tc:
        # schedule perturbation for compile-retry (Tile scheduler flake)
        for _ in range(attempt):
            nc.sync.nop(nofuse=True)
        with (
            tc.tile_pool(name="const", bufs=1) as constp,
            tc.tile_pool(name="xin", bufs=1) as xinp,
            tc.tile_pool(name="p2s", bufs=1) as p2sp,
            tc.tile_pool(name="p2a", bufs=1) as p2ap,
            tc.tile_pool(name="outs", bufs=1) as outsp,
            tc.tile_pool(name="psd", bufs=1, space="PSUM") as psd,
            tc.tile_pool(name="ps1", bufs=4, space="PSUM") as ps1,
            tc.tile_pool(name="ps2", bufs=3, space="PSUM") as ps2,
        ):
            in_dmas = []
            wk_t = constp.tile([C, WKC], BF16, tag="wk")
            wk_ap = wk_d.ap()
            # critical weights (w1 + w2) on a small fast HWDGE DMA; the
            # block-diag blocks (needed only mid-kernel) via SWDGE so the
            # 8 HWDGE queues stay exclusive (no queue-reuse WAW waits).
            in_dmas.append(nc.sync.dma_start(wk_t[:, 0:BDOFF], wk_ap[:, 0:BDOFF]))
            wkbd_dma = nc.gpsimd.dma_start(wk_t[:, BDOFF:], wk_ap[:, BDOFF:])
            w1_t = wk_t[:, 0:W2OFF].rearrange("c (kh m) -> c kh m", kh=KH)
            w2_t = wk_t[0:M1, W2OFF:BDOFF]  # [112, 128] = tile(w45.T, (7,1))
            # block-diag stage-2 weights: wbd[g] zero except rows 16g:16g+16
            wbd_t = wk_t[0:M1, BDOFF:].rearrange("p (g o) -> p g o", g=KW)

            xp_t = xinp.tile([C, WP, RP], BF16, tag="xp")
            xp_ap = xp_d.ap()
            for w0, wc in XP_PC:
                in_dmas.append(
                    nc.sync.dma_start(
                        xp_t[:, w0 : w0 + wc, :], xp_ap[:, w0 : w0 + wc, :]
                    )
                )


            p2s_t = p2sp.tile([M1, WP, H], BF16)
            p2a_t = p2ap.tile([M1, WSPLIT, H], BF16)
            outsb_t = outsp.tile([C, W, H], BF16)
            out_ap = out_d.ap()

            # dummy matmul absorbing the wk-DMA queue tick (PE single-wait)
            dt = psd.tile([1, 504], F32, tag="dummy")
            wk_dummy = nc.tensor.matmul(
                dt[:, 0:1], wk_t[0:M1, 0:1], wk_t[0:M1, 0:1], start=True, stop=True
            )


            # ---- stage 1: 9 chunks x 5 kh taps ----
            # all stage-1 evacs on ONE engine (DVE) so each shift DMA's wait
            # collapses to a single monotonic semaphore value.
            last_s1_mm = None
            for k, (w0, wc) in enumerate(S1_CH):
                pt = ps1.tile([M1, wc, H], F32, tag="p1")
                for kh in range(KH):
                    last_s1_mm = nc.tensor.matmul(
                        pt[:],
                        w1_t[:, kh, :],
                        xp_t[:, w0 : w0 + wc, DH * kh : DH * kh + H],
                        start=(kh == 0),
                        stop=(kh == KH - 1),
                    )
                nc.vector.tensor_copy(p2s_t[:, w0 : w0 + wc, :], pt[:])

            # absorb in-DMA completion ticks into SP program order BEFORE the
            # shift DMAs: the 8 physical HWDGE queues round-robin, so shifts
            # reuse in-DMA queues; covering those ticks here removes the WAW
            # queue wait from the single-wait-slot shift DMAs.
            for d in in_dmas:
                nop = nc.sync.nop(nofuse=True)
                add_dep_helper(nop.ins, d.ins, sync=True, reason="absorb in tick")

            # ---- shift DMAs (issue as soon as their p2s chunks land) ----
            # g=0..4 on the Pool SWDGE queue (ONE completion semaphore for all
            # five; ~1us software descriptor-gen each on the otherwise-idle
            # Pool engine, started early and overlapped with stage 1). g=5,6
            # (gated on the last-needed stage-1 chunk) on the SP HWDGE path.
            shift_sw = []
            for g in range(4):
                shift_sw.append(
                    nc.gpsimd.dma_start(
                        p2a_t[16 * g : 16 * g + 16, :, :],
                        p2s_t[16 * g : 16 * g + 16, 3 * g : 3 * g + WSPLIT, :],
                    )
                )
            shift_hw = []
            for g in (4, 5, 6):
                shift_hw.append(
                    nc.sync.dma_start(
                        p2a_t[16 * g : 16 * g + 16, :, :],
                        p2s_t[16 * g : 16 * g + 16, 3 * g : 3 * g + WSPLIT, :],
                    )
                )
            shift_dmas = shift_sw + shift_hw
            out_dmas = []
            last_mm = None
            last_cp = None

            def out_dma(dram_slice, sbuf_slice, evac):
                # SP HWDGE, fresh queue (no WAW): carries only the Act-evac
                # semaphore wait.
                out_dmas.append(nc.sync.dma_start(dram_slice, sbuf_slice))

            # PE dummy absorbing the SWDGE wkbd-weights tick before the
            # block-diag matmuls need them (1 wait; met long before BD).
            bd_dummy = nc.tensor.matmul(
                dt[:, 0:1], wk_t[0:M1, 0:1], wk_t[0:M1, 0:1], start=True, stop=True
            )
            add_dep_helper(
                bd_dummy.ins, last_s1_mm.ins, sync=True, reason="after stage 1"
            )
            add_dep_helper(
                bd_dummy.ins, wkbd_dma.ins, sync=True, reason="absorb wkbd tick"
            )

            # ---- stage 2a: block-diag tail (w >= WSPLIT), no DMA dep ----
            for j, (w0, wc) in enumerate(BD_CH):
                qt = ps2.tile([C, wc, H], F32, tag="p2")
                for g in range(KW):
                    last_mm = nc.tensor.matmul(
                        qt[:],
                        wbd_t[:, g, :],
                        p2s_t[:, w0 + 3 * g : w0 + 3 * g + wc, :],
                        start=(g == 0),
                        stop=(g == KW - 1),
                    )
                last_cp = nc.scalar.copy(outsb_t[:, w0 : w0 + wc, :], qt[:])
            out_dma(out_ap[:, WSPLIT * H :], outsb_t[:, WSPLIT:, :], last_cp)

            # ---- PE dummies absorbing the 7 shift-DMA completion ticks ----
            # One 1-col matmul per shift DMA (matmul = 1 wait slot; each
            # SWDGE DMA gets its OWN DMASW sem, so all 7 need absorbing),
            # ordered after the block-diag tail on PE, so stage-2b matmuls'
            # p2a deps are covered by PE program order.
            prev_mm = last_mm
            for d in shift_dmas:
                dmm = nc.tensor.matmul(
                    dt[:, 0:1], wk_t[0:M1, 0:1], wk_t[0:M1, 0:1], start=True, stop=True
                )
                add_dep_helper(dmm.ins, prev_mm.ins, sync=True, reason="after BD")
                add_dep_helper(dmm.ins, d.ins, sync=True, reason="absorb shift tick")
                prev_mm = dmm

            # ---- stage 2b: shifted single-matmul chunks (w < WSPLIT) ----
            for j, (w0, wc) in enumerate(S2_CH):
                qt = ps2.tile([C, wc, H], F32, tag="p2")
                last_mm = nc.tensor.matmul(
                    qt[:], w2_t[:], p2a_t[:, w0 : w0 + wc, :], start=True, stop=True
                )
                add_dep_helper(
                    last_mm.ins, prev_mm.ins, sync=True, reason="after shift dummies"
                )
                prev_mm = last_mm
                # s2b evacs on DVE (idle after stage 1) so PE isn't throttled
                # by Act evac latency via PSUM-bank WAR, and Act is free.
                last_cp = nc.vector.tensor_copy(outsb_t[:, w0 : w0 + wc, :], qt[:])
                if j == 3:  # w 0..31 done
                    out_dma(out_ap[:, 0 : 32 * H], outsb_t[:, 0:32, :], last_cp)
                elif j == 4:  # w 32..39 done: small last piece via SWDGE
                    out_dmas.append(
                        nc.gpsimd.dma_start(
                            out_ap[:, 32 * H : WSPLIT * H], outsb_t[:, 32:WSPLIT, :]
                        )
                    )

            # absorb DMA/engine completion ticks into SP program order so the
            # kernel-tail Drain needs no (or one) semaphore wait per proc.
            # (in_dmas already absorbed above.)
            for dep in shift_dmas + out_dmas + [wkbd_dma, last_mm, last_cp]:
                nop = nc.sync.nop(nofuse=True)
                add_dep_helper(nop.ins, dep.ins, sync=True, reason="absorb tick")
    return nc


def _get_nc():
    global _NC
    if _NC is None:
        _NC = _build_nc()
    return _NC


def _prep_inputs(x, w3, w4, w5):
    w45 = (w5.astype(np.float64) @ w4.astype(np.float64)).astype(np.float32)
    # w1[c, kh, kw*CO+co] = w3[co, c, kh, kw]
    w1 = np.transpose(w3, (1, 2, 3, 0)).reshape(C, KH * M1)
    # w2[kw*CO+co, o] = w45[o, co]
    w2 = np.tile(w45.T, (KW, 1))  # [112, 128]
    wk = np.zeros((C, WKC), np.float32)
    wk[:, :W2OFF] = w1
    wk[:M1, W2OFF:BDOFF] = w2
    for g in range(KW):
        wk[16 * g : 16 * g + 16, BDOFF + 128 * g : BDOFF + 128 * (g + 1)] = w45.T
    wk = wk.astype(ml_dtypes.bfloat16)
    # xp[c, w', r] = xpad[c, r, w']  (padded, W-major)
    xp = np.zeros((N, C, WP, RP), np.float32)
    xp[:, :, PW : PW + W, PH : PH + H] = np.transpose(x, (0, 1, 3, 2))
    xp = xp.astype(ml_dtypes.bfloat16)
    return xp, wk


def kernel(x, w3, w4, w5, trace=False):
    x = np.asarray(x, np.float32)
    w3 = np.asarray(w3, np.float32)
    w4 = np.asarray(w4, np.float32)
    w5 = np.asarray(w5, np.float32)
    xp, wk = _prep_inputs(x, w3, w4, w5)
    in_maps = [{"xp": np.ascontiguousarray(xp[n]), "wk": wk} for n in range(N)]
    global _NC
    res = None
    last_err = None
    for attempt in range(6):
        if _NC is None:
            _NC = _build_nc(attempt)
        try:
            res = run_bass_kernel_spmd(
                _NC, in_maps, core_ids=list(range(N)), trace=trace
            )
            break
        except Exception as e:  # compile-schedule flake: rebuild perturbed
            last_err = e
            _NC = None
    if res is None:
        raise last_err
    # device output is [C, w, h] bf16 -> [C, h, w] f32
    out = np.stack(
        [
            np.transpose(
                np.asarray(res.results[n]["out"]).astype(np.float32).reshape(C, W, H),
                (0, 2, 1),
            )
            for n in range(N)
        ]
    )
    if trace:
        return out, res
    return out


# revision 36
# speedup vs baseline: 1.1037x; 1.1037x over previous
"""Trainium2 Bass kernel for dilated 5x7 conv (128->16ch) + 1x1 (16->16) + 1x1 (16->128).

Strategy (data-parallel, 1 image per core across 8 cores):
  reference: y = conv_dilated(x, w3, dil=(2,3), pad=(4,9)); y = w4@y; y = w5@y
  Host folds w45 = w5 @ w4  [128, 16].

  Per core, image x [128, 56, 56] zero-padded AND W-major transposed to
  xp [c=128, w'=74, r=64] (bf16), xp[c, w', r] = xpad[c, r, w'].

  Stage 1 (TensorE, contract kh): for each kh in 0..4, matmul with
      lhsT = w1[:, kh, :] [c=128, (kw,co)=112], rhs = xp[:, wchunk, 2kh:2kh+56],
      PSUM-accumulating over kh  ->  P[(kw,co), w', h] =
      sum_{kh,c} w3[co,c,kh,kw] * xpad[c, h+2kh, w'].
  Evacuate PSUM->SBUF p2s [112, 74, 56] bf16 (w-major => any w-window of all
  h is CONTIGUOUS in the free dim).
  Shift-align for w<WSPLIT via SBUF->SBUF DMA (contiguous 16-partition runs):
      p2a[(kw,co), w, h] = p2s[(kw,co), w+3kw, h].
  Stage 2a (w in [WSPLIT, 56), block-diag, no shift/DMA dependency):
      out[o, w, h] += sum_co w45[o,co] * p2s[(g,co), w+3g, h]   (7 K=16 matmuls)
  Stage 2b (w in [0, WSPLIT), single matmul per chunk, K=112):
      out[o, w, h] = sum_{(kw,co)} w45[o,co] * p2a[(kw,co), w, h].
  Evacuate (RR engines) to bf16, DMA out; host casts f32 + transposes (w,h)->(h,w).
"""

import os
import sys

import numpy as np

for _p in ("/opt/trn_rl_repo", "/root/.axon_site/_ro/trn_rl_repo"):
    if os.path.isdir(_p) and _p not in sys.path:
        sys.path.insert(0, _p)

import ml_dtypes  # noqa: E402

import concourse.bass as bass  # noqa: E402
import concourse.tile as tile  # noqa: E402
from concourse.tile_rust import add_dep_helper  # noqa: E402
from concourse import mybir  # noqa: E402
from concourse.bass_utils import run_bass_kernel_spmd  # noqa: E402

N, C, H, W = 8, 128, 56, 56
CO = 16
KH, KW = 5, 7
DH, DW = 2, 3
PH, PW = 4, 9
RP, WP = H + 2 * PH, W + 2 * PW  # 64 padded rows, 74 padded cols
M1 = KW * CO  # 112 = (kw, co)
WSPLIT = 40  # w < WSPLIT via shift-DMA + single matmul; w >= WSPLIT block-diag
# stage-1 w' chunks (PSUM bank: <=512 fp32/partition => <=9 w' of 56 h)
S1_CH = [(0, 9), (9, 9), (18, 9), (27, 9), (36, 9), (45, 9), (54, 9), (63, 9), (72, 2)]
# shift DMAs need p2s w' <= 3*6 + WSPLIT - 1 = 57 -> stage-1 chunks 0..6
# xp input DMA pieces: boundaries land on stage-1 chunks 0 and 3, which use
# FRESH ps1 buffers (bufs=4) -> each boundary matmul carries only the DMA
# wait, never DMA wait + PSUM-bank-WAR wait (matmul has ONE wait slot).
# Total HWDGE DMAs = wk1 + 3 xp + 2 shifts + 2 outs = 8 = #physical queues,
# so NO queue is reused and NO DMA ever carries a queue-WAW wait. First xp
# piece is tiny so stage-1 chunk 0 starts early; boundaries at chunks 0,1,3
# all use fresh ps1 buffers (bufs=4).
XP_PC = [(0, 9), (9, 18), (27, 47)]
BD_CH = [(40, 8), (48, 8)]  # block-diag stage-2 w chunks
S2_CH = [(0, 8), (8, 8), (16, 8), (24, 8), (32, 8)]  # shifted stage-2 w chunks
# out DMA pieces (w ranges), in issue order; last computed piece is small
# wk free cols: w1 (5*112) + w2 (128) + 7 zero-padded block-diag w2 blocks
W2OFF = KH * M1  # 560
BDOFF = W2OFF + 128  # 688
WKC = BDOFF + KW * 128  # 1584
BF16 = mybir.dt.bfloat16
F32 = mybir.dt.float32

_NC = None


def _build_nc(attempt=0):
    nc = bass.Bass(
        "TRN2",
        target_bir_lowering=False,
        debug=False,
        enable_asserts=False,
        num_devices=N,
    )
    xp_d = nc.dram_tensor("xp", [C, WP, RP], BF16, kind="ExternalInput")
    wk_d = nc.dram_tensor("wk", [C, WKC], BF16, kind="ExternalInput")
    out_d = nc.dram_tensor("out", [C, W * H], BF16, kind="ExternalOutput")

    with tile.TileContext(nc) as tc:
        # schedule perturbation for compile-retry (Tile scheduler flake)
        for _ in range(attempt):
            nc.sync.nop(nofuse=True)
        with (
            tc.tile_pool(name="const", bufs=1) as constp,
            tc.tile_pool(name="xin", bufs=1) as xinp,
            tc.tile_pool(name="p2s", bufs=1) as p2sp,
            tc.tile_pool(name="p2a", bufs=1) as p2ap,
            tc.tile_pool(name="outs", bufs=1) as outsp,
            tc.tile_pool(name="psd", bufs=1, space="PSUM") as psd,
            tc.tile_pool(name="ps1", bufs=4, space="PSUM") as ps1,
            tc.tile_pool(name="ps2", bufs=3, space="PSUM") as ps2,
        ):
            in_dmas = []
            wk_t = constp.tile([C, WKC], BF16, tag="wk")
            wk_ap = wk_d.ap()
            # critical weights (w1 + w2) on a small fast HWDGE DMA; the
            # block-diag blocks (needed only mid-kernel) via SWDGE so the
            # 8 HWDGE queues stay exclusive (no queue-reuse WAW waits).
            in_dmas.append(nc.sync.dma_start(wk_t[:, 0:BDOFF], wk_ap[:, 0:BDOFF]))
            wkbd_dma = nc.gpsimd.dma_start(wk_t[:, BDOFF:], wk_ap[:, BDOFF:])
            w1_t = wk_t[:, 0:W2OFF].rearrange("c (kh m) -> c kh m", kh=KH)
            w2_t = wk_t[0:M1, W2OFF:BDOFF]  # [112, 128] = tile(w45.T, (7,1))
            # block-diag stage-2 weights: wbd[g] zero except rows 16g:16g+16
            wbd_t = wk_t[0:M1, BDOFF:].rearrange("p (g o) -> p g o", g=KW)

            xp_t = xinp.tile([C, WP, RP], BF16, tag="xp")
            xp_ap = xp_d.ap()
            for w0, wc in XP_PC:
                in_dmas.append(
                    nc.sync.dma_start(
                        xp_t[:, w0 : w0 + wc, :], xp_ap[:, w0 : w0 + wc, :]
                    )
                )


            p2s_t = p2sp.tile([M1, WP, H], BF16)
            p2a_t = p2ap.tile([M1, WSPLIT, H], BF16)
            outsb_t = outsp.tile([C, W, H], BF16)
            out_ap = out_d.ap()

            # dummy matmul absorbing the wk-DMA queue tick (PE single-wait)
            dt = psd.tile([1, 504], F32, tag="dummy")
            wk_dummy = nc.tensor.matmul(
                dt[:, 0:1], wk_t[0:M1, 0:1], wk_t[0:M1, 0:1], start=True, stop=True
            )


            # ---- stage 1: 9 chunks x 5 kh taps ----
            # all stage-1 evacs on ONE engine (DVE) so each shift DMA's wait
            # collapses to a single monotonic semaphore value.
            last_s1_mm = None
            for k, (w0, wc) in enumerate(S1_CH):
                pt = ps1.tile([M1, wc, H], F32, tag="p1")
                for kh in range(KH):
                    last_s1_mm = nc.tensor.matmul(
                        pt[:],
                        w1_t[:, kh, :],
                        xp_t[:, w0 : w0 + wc, DH * kh : DH * kh + H],
                        start=(kh == 0),
                        stop=(kh == KH - 1),
                    )
                nc.vector.tensor_copy(p2s_t[:, w0 : w0 + wc, :], pt[:])

            # absorb in-DMA completion ticks into SP program order BEFORE the
            # shift DMAs: the 8 physical HWDGE queues round-robin, so shifts
            # reuse in-DMA queues; covering those ticks here removes the WAW
            # queue wait from the single-wait-slot shift DMAs.
            for d in in_dmas:
                nop = nc.sync.nop(nofuse=True)
                add_dep_helper(nop.ins, d.ins, sync=True, reason="absorb in tick")

            # ---- shift DMAs (issue as soon as their p2s chunks land) ----
            # g=0..4 on the Pool SWDGE queue (ONE completion semaphore for all
            # five; ~1us software descriptor-gen each on the otherwise-idle
            # Pool engine, started early and overlapped with stage 1). g=5,6
            # (gated on the last-needed stage-1 chunk) on the SP HWDGE path.
            shift_sw = []
            for g in range(5):
                shift_sw.append(
                    nc.gpsimd.dma_start(
                        p2a_t[16 * g : 16 * g + 16, :, :],
                        p2s_t[16 * g : 16 * g + 16, 3 * g : 3 * g + WSPLIT, :],
                    )
                )
            shift_hw = []
            for g in (5, 6):
                shift_hw.append(
                    nc.sync.dma_start(
                        p2a_t[16 * g : 16 * g + 16, :, :],
                        p2s_t[16 * g : 16 * g + 16, 3 * g : 3 * g + WSPLIT, :],
                    )
                )
            shift_dmas = shift_sw + shift_hw
            out_dmas = []
            last_mm = None
            last_cp = None

            def out_dma(dram_slice, sbuf_slice, evac):
                # SP HWDGE, fresh queue (no WAW): carries only the Act-evac
                # semaphore wait.
                out_dmas.append(nc.sync.dma_start(dram_slice, sbuf_slice))

            # PE dummy absorbing the SWDGE wkbd-weights tick before the
            # block-diag matmuls need them (1 wait; met long before BD).
            bd_dummy = nc.tensor.matmul(
                dt[:, 0:1], wk_t[0:M1, 0:1], wk_t[0:M1, 0:1], start=True, stop=True
            )
            add_dep_helper(
                bd_dummy.ins, last_s1_mm.ins, sync=True, reason="after stage 1"
            )
            add_dep_helper(
                bd_dummy.ins, wkbd_dma.ins, sync=True, reason="absorb wkbd tick"
            )

            # ---- stage 2a: block-diag tail (w >= WSPLIT), no DMA dep ----
            for j, (w0, wc) in enumerate(BD_CH):
                qt = ps2.tile([C, wc, H], F32, tag="p2")
                for g in range(KW):
                    last_mm = nc.tensor.matmul(
                        qt[:],
                        wbd_t[:, g, :],
                        p2s_t[:, w0 + 3 * g : w0 + 3 * g + wc, :],
                        start=(g == 0),
                        stop=(g == KW - 1),
                    )
                last_cp = nc.scalar.copy(outsb_t[:, w0 : w0 + wc, :], qt[:])
            out_dma(out_ap[:, WSPLIT * H :], outsb_t[:, WSPLIT:, :], last_cp)

            # ---- PE dummies absorbing the 7 shift-DMA completion ticks ----
            # One 1-col matmul per shift DMA (matmul = 1 wait slot; each
            # SWDGE DMA gets its OWN DMASW sem, so all 7 need absorbing),
            # ordered after the block-diag tail on PE, so stage-2b matmuls'
            # p2a deps are covered by PE program order.
            prev_mm = last_mm
            for d in shift_dmas:
                dmm = nc.tensor.matmul(
                    dt[:, 0:1], wk_t[0:M1, 0:1], wk_t[0:M1, 0:1], start=True, stop=True
                )
                add_dep_helper(dmm.ins, prev_mm.ins, sync=True, reason="after BD")
                add_dep_helper(dmm.ins, d.ins, sync=True, reason="absorb shift tick")
                prev_mm = dmm

            # ---- stage 2b: shifted single-matmul chunks (w < WSPLIT) ----
            for j, (w0, wc) in enumerate(S2_CH):
                qt = ps2.tile([C, wc, H], F32, tag="p2")
                last_mm = nc.tensor.matmul(
                    qt[:], w2_t[:], p2a_t[:, w0 : w0 + wc, :], start=True, stop=True
                )
                add_dep_helper(
                    last_mm.ins, prev_mm.ins, sync=True, reason="after shift dummies"
                )
                prev_mm = last_mm
                # s2b evacs on DVE (idle after stage 1) so PE isn't throttled
                # by Act evac latency via PSUM-bank WAR, and Act is free.
                last_cp = nc.vector.tensor_copy(outsb_t[:, w0 : w0 + wc, :], qt[:])
                if j == 3:  # w 0..31 done
                    out_dma(out_ap[:, 0 : 32 * H], outsb_t[:, 0:32, :], last_cp)
                elif j == 4:  # w 32..39 done: small last piece via SWDGE
                    out_dmas.append(
                        nc.gpsimd.dma_start(
                            out_ap[:, 32 * H : WSPLIT * H], outsb_t[:, 32:WSPLIT, :]
                        )
                    )

            # absorb DMA/engine completion ticks into SP program order so the
            # kernel-tail Drain needs no (or one) semaphore wait per proc.
            # (in_dmas already absorbed above.)
            for dep in shift_dmas + out_dmas + [wkbd_dma, last_mm, last_cp]:
                nop = nc.sync.nop(nofuse=True)
                add_dep_helper(nop.ins, dep.ins, sync=True, reason="absorb tick")
    return nc


def _get_nc():
    global _NC
    if _NC is None:
        _NC = _build_nc()
    return _NC


def _prep_inputs(x, w3, w4, w5):
    w45 = (w5.astype(np.float64) @ w4.astype(np.float64)).astype(np.float32)
    # w1[c, kh, kw*CO+co] = w3[co, c, kh, kw]
    w1 = np.transpose(w3, (1, 2, 3, 0)).reshape(C, KH * M1)
    # w2[kw*CO+co, o] = w45[o, co]
    w2 = np.tile(w45.T, (KW, 1))  # [112, 128]
    wk = np.zeros((C, WKC), np.float32)
    wk[:, :W2OFF] = w1
    wk[:M1, W2OFF:BDOFF] = w2
    for g in range(KW):
        wk[16 * g : 16 * g + 16, BDOFF + 128 * g : BDOFF + 128 * (g + 1)] = w45.T
    wk = wk.astype(ml_dtypes.bfloat16)
    # xp[c, w', r] = xpad[c, r, w']  (padded, W-major)
    xp = np.zeros((N, C, WP, RP), np.float32)
    xp[:, :, PW : PW + W, PH : PH + H] = np.transpose(x, (0, 1, 3, 2))
    xp = xp.astype(ml_dtypes.bfloat16)
    return xp, wk


def kernel(x, w3, w4, w5, trace=False):
    x = np.asarray(x, np.float32)
    w3 = np.asarray(w3, np.float32)
    w4 = np.asarray(w4, np.float32)
    w5 = np.asarray(w5, np.float32)
    xp, wk = _prep_inputs(x, w3, w4, w5)
    in_maps = [{"xp": np.ascontiguousarray(xp[n]), "wk": wk} for n in range(N)]
    global _NC
    res = None
    last_err = None
    for attempt in range(6):
        if _NC is None:
            _NC = _build_nc(attempt)
        try:
            res = run_bass_kernel_spmd(
                _NC, in_maps, core_ids=list(range(N)), trace=trace
            )
            break
        except Exception as e:  # compile-schedule flake: rebuild perturbed
            last_err = e
            _NC = None
    if res is None:
        raise last_err
    # device output is [C, w, h] bf16 -> [C, h, w] f32
    out = np.stack(
        [
            np.transpose(
                np.asarray(res.results[n]["out"]).astype(np.float32).reshape(C, W, H),
                (0, 2, 1),
            )
            for n in range(N)
        ]
    )
    if trace:
        return out, res
    return out


# revision 37
# speedup vs baseline: 1.1081x; 1.0040x over previous
"""Trainium2 Bass kernel for dilated 5x7 conv (128->16ch) + 1x1 (16->16) + 1x1 (16->128).

Strategy (data-parallel, 1 image per core across 8 cores):
  reference: y = conv_dilated(x, w3, dil=(2,3), pad=(4,9)); y = w4@y; y = w5@y
  Host folds w45 = w5 @ w4  [128, 16].

  Per core, image x [128, 56, 56] zero-padded AND W-major transposed to
  xp [c=128, w'=74, r=64] (bf16), xp[c, w', r] = xpad[c, r, w'].

  Stage 1 (TensorE, contract kh): for each kh in 0..4, matmul with
      lhsT = w1[:, kh, :] [c=128, (kw,co)=112], rhs = xp[:, wchunk, 2kh:2kh+56],
      PSUM-accumulating over kh  ->  P[(kw,co), w', h] =
      sum_{kh,c} w3[co,c,kh,kw] * xpad[c, h+2kh, w'].
  Evacuate PSUM->SBUF p2s [112, 74, 56] bf16 (w-major => any w-window of all
  h is CONTIGUOUS in the free dim).
  Shift-align for w<WSPLIT via SBUF->SBUF DMA (contiguous 16-partition runs):
      p2a[(kw,co), w, h] = p2s[(kw,co), w+3kw, h].
  Stage 2a (w in [WSPLIT, 56), block-diag, no shift/DMA dependency):
      out[o, w, h] += sum_co w45[o,co] * p2s[(g,co), w+3g, h]   (7 K=16 matmuls)
  Stage 2b (w in [0, WSPLIT), single matmul per chunk, K=112):
      out[o, w, h] = sum_{(kw,co)} w45[o,co] * p2a[(kw,co), w, h].
  Evacuate (RR engines) to bf16, DMA out; host casts f32 + transposes (w,h)->(h,w).
"""

import os
import sys

import numpy as np

for _p in ("/opt/trn_rl_repo", "/root/.axon_site/_ro/trn_rl_repo"):
    if os.path.isdir(_p) and _p not in sys.path:
        sys.path.insert(0, _p)

import ml_dtypes  # noqa: E402

import concourse.bass as bass  # noqa: E402
import concourse.tile as tile  # noqa: E402
from concourse.tile_rust import add_dep_helper  # noqa: E402
from concourse import mybir  # noqa: E402
from concourse.bass_utils import run_bass_kernel_spmd  # noqa: E402

N, C, H, W = 8, 128, 56, 56
CO = 16
KH, KW = 5, 7
DH, DW = 2, 3
PH, PW = 4, 9
RP, WP = H + 2 * PH, W + 2 * PW  # 64 padded rows, 74 padded cols
M1 = KW * CO  # 112 = (kw, co)
WSPLIT = 40  # w < WSPLIT via shift-DMA + single matmul; w >= WSPLIT block-diag
# stage-1 w' chunks (PSUM bank: <=512 fp32/partition => <=9 w' of 56 h)
S1_CH = [(0, 9), (9, 9), (18, 9), (27, 9), (36, 9), (45, 9), (54, 9), (63, 9), (72, 2)]
# shift DMAs need p2s w' <= 3*6 + WSPLIT - 1 = 57 -> stage-1 chunks 0..6
# xp input DMA pieces: boundaries land on stage-1 chunks 0 and 3, which use
# FRESH ps1 buffers (bufs=4) -> each boundary matmul carries only the DMA
# wait, never DMA wait + PSUM-bank-WAR wait (matmul has ONE wait slot).
# Total HWDGE DMAs = wk1 + 3 xp + 2 shifts + 2 outs = 8 = #physical queues,
# so NO queue is reused and NO DMA ever carries a queue-WAW wait. First xp
# piece is tiny so stage-1 chunk 0 starts early; boundaries at chunks 0,1,3
# all use fresh ps1 buffers (bufs=4).
XP_PC = [(0, 9), (9, 18), (27, 47)]
BD_CH = [(40, 8), (48, 8)]  # block-diag stage-2 w chunks
S2_CH = [(0, 8), (8, 8), (16, 8), (24, 8), (32, 8)]  # shifted stage-2 w chunks
# out DMA pieces (w ranges), in issue order; last computed piece is small
# wk free cols: w1 (5*112) + w2 (128) + 7 zero-padded block-diag w2 blocks
W2OFF = KH * M1  # 560
BDOFF = W2OFF + 128  # 688
WKC = BDOFF + KW * 128  # 1584
BF16 = mybir.dt.bfloat16
F32 = mybir.dt.float32

_NC = None


def _build_nc(attempt=0):
    nc = bass.Bass(
        "TRN2",
        target_bir_lowering=False,
        debug=False,
        enable_asserts=False,
        num_devices=N,
    )
    xp_d = nc.dram_tensor("xp", [C, WP, RP], BF16, kind="ExternalInput")
    wk_d = nc.dram_tensor("wk", [C, WKC], BF16, kind="ExternalInput")
    out_d = nc.dram_tensor("out", [C, W * H], BF16, kind="ExternalOutput")

    with tile.TileContext(nc) as tc:
        # schedule perturbation for compile-retry (Tile scheduler flake)
        for _ in range(attempt):
            nc.sync.nop(nofuse=True)
        with (
            tc.tile_pool(name="const", bufs=1) as constp,
            tc.tile_pool(name="xin", bufs=1) as xinp,
            tc.tile_pool(name="p2s", bufs=1) as p2sp,
            tc.tile_pool(name="p2a", bufs=1) as p2ap,
            tc.tile_pool(name="outs", bufs=1) as outsp,
            tc.tile_pool(name="psd", bufs=1, space="PSUM") as psd,
            tc.tile_pool(name="ps1", bufs=4, space="PSUM") as ps1,
            tc.tile_pool(name="ps2", bufs=3, space="PSUM") as ps2,
        ):
            in_dmas = []
            wk_t = constp.tile([C, WKC], BF16, tag="wk")
            wk_ap = wk_d.ap()
            xp_t = xinp.tile([C, WP, RP], BF16, tag="xp")
            xp_ap = xp_d.ap()
            # xp piece 0 issued FIRST (it gates stage-1 chunk 0); then the
            # critical weights (w1 + w2) on a small fast HWDGE DMA; the
            # block-diag blocks (needed only mid-kernel) via SWDGE so the
            # 8 HWDGE queues stay exclusive (no queue-reuse WAW waits).
            w0, wc = XP_PC[0]
            in_dmas.append(
                nc.sync.dma_start(xp_t[:, w0 : w0 + wc, :], xp_ap[:, w0 : w0 + wc, :])
            )
            in_dmas.append(nc.sync.dma_start(wk_t[:, 0:BDOFF], wk_ap[:, 0:BDOFF]))
            wkbd_dma = nc.gpsimd.dma_start(wk_t[:, BDOFF:], wk_ap[:, BDOFF:])
            w1_t = wk_t[:, 0:W2OFF].rearrange("c (kh m) -> c kh m", kh=KH)
            w2_t = wk_t[0:M1, W2OFF:BDOFF]  # [112, 128] = tile(w45.T, (7,1))
            # block-diag stage-2 weights: wbd[g] zero except rows 16g:16g+16
            wbd_t = wk_t[0:M1, BDOFF:].rearrange("p (g o) -> p g o", g=KW)

            for w0, wc in XP_PC[1:]:
                in_dmas.append(
                    nc.sync.dma_start(
                        xp_t[:, w0 : w0 + wc, :], xp_ap[:, w0 : w0 + wc, :]
                    )
                )


            p2s_t = p2sp.tile([M1, WP, H], BF16)
            p2a_t = p2ap.tile([M1, WSPLIT, H], BF16)
            outsb_t = outsp.tile([C, W, H], BF16)
            out_ap = out_d.ap()

            # dummy matmul absorbing the wk-DMA queue tick (PE single-wait)
            dt = psd.tile([1, 504], F32, tag="dummy")
            wk_dummy = nc.tensor.matmul(
                dt[:, 0:1], wk_t[0:M1, 0:1], wk_t[0:M1, 0:1], start=True, stop=True
            )


            # ---- stage 1: 9 chunks x 5 kh taps ----
            # all stage-1 evacs on ONE engine (DVE) so each shift DMA's wait
            # collapses to a single monotonic semaphore value.
            last_s1_mm = None
            for k, (w0, wc) in enumerate(S1_CH):
                pt = ps1.tile([M1, wc, H], F32, tag="p1")
                for kh in range(KH):
                    last_s1_mm = nc.tensor.matmul(
                        pt[:],
                        w1_t[:, kh, :],
                        xp_t[:, w0 : w0 + wc, DH * kh : DH * kh + H],
                        start=(kh == 0),
                        stop=(kh == KH - 1),
                    )
                nc.vector.tensor_copy(p2s_t[:, w0 : w0 + wc, :], pt[:])

            # absorb in-DMA completion ticks into SP program order BEFORE the
            # shift DMAs: the 8 physical HWDGE queues round-robin, so shifts
            # reuse in-DMA queues; covering those ticks here removes the WAW
            # queue wait from the single-wait-slot shift DMAs.
            for d in in_dmas:
                nop = nc.sync.nop(nofuse=True)
                add_dep_helper(nop.ins, d.ins, sync=True, reason="absorb in tick")

            # ---- shift DMAs (issue as soon as their p2s chunks land) ----
            # g=0..4 on the Pool SWDGE queue (ONE completion semaphore for all
            # five; ~1us software descriptor-gen each on the otherwise-idle
            # Pool engine, started early and overlapped with stage 1). g=5,6
            # (gated on the last-needed stage-1 chunk) on the SP HWDGE path.
            shift_sw = []
            for g in range(5):
                shift_sw.append(
                    nc.gpsimd.dma_start(
                        p2a_t[16 * g : 16 * g + 16, :, :],
                        p2s_t[16 * g : 16 * g + 16, 3 * g : 3 * g + WSPLIT, :],
                    )
                )
            shift_hw = []
            for g in (5, 6):
                shift_hw.append(
                    nc.sync.dma_start(
                        p2a_t[16 * g : 16 * g + 16, :, :],
                        p2s_t[16 * g : 16 * g + 16, 3 * g : 3 * g + WSPLIT, :],
                    )
                )
            shift_dmas = shift_sw + shift_hw
            out_dmas = []
            last_mm = None
            last_cp = None

            def out_dma(dram_slice, sbuf_slice, evac):
                # SP HWDGE, fresh queue (no WAW): carries only the Act-evac
                # semaphore wait.
                out_dmas.append(nc.sync.dma_start(dram_slice, sbuf_slice))

            # PE dummy absorbing the SWDGE wkbd-weights tick before the
            # block-diag matmuls need them (1 wait; met long before BD).
            bd_dummy = nc.tensor.matmul(
                dt[:, 0:1], wk_t[0:M1, 0:1], wk_t[0:M1, 0:1], start=True, stop=True
            )
            add_dep_helper(
                bd_dummy.ins, last_s1_mm.ins, sync=True, reason="after stage 1"
            )
            add_dep_helper(
                bd_dummy.ins, wkbd_dma.ins, sync=True, reason="absorb wkbd tick"
            )

            # ---- stage 2a: block-diag tail (w >= WSPLIT), no DMA dep ----
            for j, (w0, wc) in enumerate(BD_CH):
                qt = ps2.tile([C, wc, H], F32, tag="p2")
                for g in range(KW):
                    last_mm = nc.tensor.matmul(
                        qt[:],
                        wbd_t[:, g, :],
                        p2s_t[:, w0 + 3 * g : w0 + 3 * g + wc, :],
                        start=(g == 0),
                        stop=(g == KW - 1),
                    )
                last_cp = nc.scalar.copy(outsb_t[:, w0 : w0 + wc, :], qt[:])
            out_dma(out_ap[:, WSPLIT * H :], outsb_t[:, WSPLIT:, :], last_cp)

            # ---- PE dummies absorbing the 7 shift-DMA completion ticks ----
            # One 1-col matmul per shift DMA (matmul = 1 wait slot; each
            # SWDGE DMA gets its OWN DMASW sem, so all 7 need absorbing),
            # ordered after the block-diag tail on PE, so stage-2b matmuls'
            # p2a deps are covered by PE program order.
            prev_mm = last_mm
            for d in shift_dmas:
                dmm = nc.tensor.matmul(
                    dt[:, 0:1], wk_t[0:M1, 0:1], wk_t[0:M1, 0:1], start=True, stop=True
                )
                add_dep_helper(dmm.ins, prev_mm.ins, sync=True, reason="after BD")
                add_dep_helper(dmm.ins, d.ins, sync=True, reason="absorb shift tick")
                prev_mm = dmm

            # ---- stage 2b: shifted single-matmul chunks (w < WSPLIT) ----
            for j, (w0, wc) in enumerate(S2_CH):
                qt = ps2.tile([C, wc, H], F32, tag="p2")
                last_mm = nc.tensor.matmul(
                    qt[:], w2_t[:], p2a_t[:, w0 : w0 + wc, :], start=True, stop=True
                )
                add_dep_helper(
                    last_mm.ins, prev_mm.ins, sync=True, reason="after shift dummies"
                )
                prev_mm = last_mm
                # s2b evacs on DVE (idle after stage 1) so PE isn't throttled
                # by Act evac latency via PSUM-bank WAR, and Act is free.
                last_cp = nc.vector.tensor_copy(outsb_t[:, w0 : w0 + wc, :], qt[:])
                if j == 3:  # w 0..31 done
                    out_dma(out_ap[:, 0 : 32 * H], outsb_t[:, 0:32, :], last_cp)
                elif j == 4:  # w 32..39 done: small last piece via SWDGE
                    out_dmas.append(
                        nc.gpsimd.dma_start(
                            out_ap[:, 32 * H : WSPLIT * H], outsb_t[:, 32:WSPLIT, :]
                        )
                    )

            # absorb DMA/engine completion ticks into SP program order so the
            # kernel-tail Drain needs no (or one) semaphore wait per proc.
            # (in_dmas already absorbed above.)
            for dep in shift_dmas + out_dmas + [wkbd_dma, last_mm, last_cp]:
                nop = nc.sync.nop(nofuse=True)
                add_dep_helper(nop.ins, dep.ins, sync=True, reason="absorb tick")
    return nc


def _get_nc():
    global _NC
    if _NC is None:
        _NC = _build_nc()
    return _NC


def _prep_inputs(x, w3, w4, w5):
    w45 = (w5.astype(np.float64) @ w4.astype(np.float64)).astype(np.float32)
    # w1[c, kh, kw*CO+co] = w3[co, c, kh, kw]
    w1 = np.transpose(w3, (1, 2, 3, 0)).reshape(C, KH * M1)
    # w2[kw*CO+co, o] = w45[o, co]
    w2 = np.tile(w45.T, (KW, 1))  # [112, 128]
    wk = np.zeros((C, WKC), np.float32)
    wk[:, :W2OFF] = w1
    wk[:M1, W2OFF:BDOFF] = w2
    for g in range(KW):
        wk[16 * g : 16 * g + 16, BDOFF + 128 * g : BDOFF + 128 * (g + 1)] = w45.T
    wk = wk.astype(ml_dtypes.bfloat16)
    # xp[c, w', r] = xpad[c, r, w']  (padded, W-major)
    xp = np.zeros((N, C, WP, RP), np.float32)
    xp[:, :, PW : PW + W, PH : PH + H] = np.transpose(x, (0, 1, 3, 2))
    xp = xp.astype(ml_dtypes.bfloat16)
    return xp, wk


def kernel(x, w3, w4, w5, trace=False):
    x = np.asarray(x, np.float32)
    w3 = np.asarray(w3, np.float32)
    w4 = np.asarray(w4, np.float32)
    w5 = np.asarray(w5, np.float32)
    xp, wk = _prep_inputs(x, w3, w4, w5)
    in_maps = [{"xp": np.ascontiguousarray(xp[n]), "wk": wk} for n in range(N)]
    global _NC
    res = None
    last_err = None
    for attempt in range(6):
        if _NC is None:
            _NC = _build_nc(attempt)
        try:
            res = run_bass_kernel_spmd(
                _NC, in_maps, core_ids=list(range(N)), trace=trace
            )
            break
        except Exception as e:  # compile-schedule flake: rebuild perturbed
            last_err = e
            _NC = None
    if res is None:
        raise last_err
    # device output is [C, w, h] bf16 -> [C, h, w] f32
    out = np.stack(
        [
            np.transpose(
                np.asarray(res.results[n]["out"]).astype(np.float32).reshape(C, W, H),
                (0, 2, 1),
            )
            for n in range(N)
        ]
    )
    if trace:
        return out, res
    return out


# revision 38
# speedup vs baseline: 1.1706x; 1.0565x over previous
"""Trainium2 Bass kernel for dilated 5x7 conv (128->16ch) + 1x1 (16->16) + 1x1 (16->128).

Strategy (data-parallel, 1 image per core across 8 cores):
  reference: y = conv_dilated(x, w3, dil=(2,3), pad=(4,9)); y = w4@y; y = w5@y
  Host folds w45 = w5 @ w4  [128, 16].

  Per core, image x [128, 56, 56] zero-padded AND W-major transposed to
  xp [c=128, w'=74, r=64] (bf16), xp[c, w', r] = xpad[c, r, w'].

  Stage 1 (TensorE, contract kh): for each kh in 0..4, matmul with
      lhsT = w1[:, kh, :] [c=128, (kw,co)=112], rhs = xp[:, wchunk, 2kh:2kh+56],
      PSUM-accumulating over kh  ->  P[(kw,co), w', h] =
      sum_{kh,c} w3[co,c,kh,kw] * xpad[c, h+2kh, w'].
  Evacuate PSUM->SBUF p2s [112, 74, 56] bf16 (w-major => any w-window of all
  h is CONTIGUOUS in the free dim).
  Shift-align for w<WSPLIT via SBUF->SBUF DMA (contiguous 16-partition runs):
      p2a[(kw,co), w, h] = p2s[(kw,co), w+3kw, h].
  Stage 2a (w in [WSPLIT, 56), block-diag, no shift/DMA dependency):
      out[o, w, h] += sum_co w45[o,co] * p2s[(g,co), w+3g, h]   (7 K=16 matmuls)
  Stage 2b (w in [0, WSPLIT), single matmul per chunk, K=112):
      out[o, w, h] = sum_{(kw,co)} w45[o,co] * p2a[(kw,co), w, h].
  Evacuate (RR engines) to bf16, DMA out; host casts f32 + transposes (w,h)->(h,w).
"""

import os
import sys

import numpy as np

for _p in ("/opt/trn_rl_repo", "/root/.axon_site/_ro/trn_rl_repo"):
    if os.path.isdir(_p) and _p not in sys.path:
        sys.path.insert(0, _p)

import ml_dtypes  # noqa: E402

import concourse.bass as bass  # noqa: E402
import concourse.tile as tile  # noqa: E402
from concourse.tile_rust import add_dep_helper  # noqa: E402
from concourse import mybir  # noqa: E402
from concourse.bass_utils import run_bass_kernel_spmd  # noqa: E402

N, C, H, W = 8, 128, 56, 56
CO = 16
KH, KW = 5, 7
DH, DW = 2, 3
PH, PW = 4, 9
RP, WP = H + 2 * PH, W + 2 * PW  # 64 padded rows, 74 padded cols
M1 = KW * CO  # 112 = (kw, co)
WSPLIT = 40  # w < WSPLIT via shift-DMA + single matmul; w >= WSPLIT block-diag
# stage-1 w' chunks (PSUM bank: <=512 fp32/partition => <=9 w' of 56 h)
S1_CH = [(0, 9), (9, 9), (18, 9), (27, 9), (36, 9), (45, 9), (54, 9), (63, 9), (72, 2)]
# shift DMAs need p2s w' <= 3*6 + WSPLIT - 1 = 57 -> stage-1 chunks 0..6
# xp input DMA pieces: boundaries land on stage-1 chunks 0 and 3, which use
# FRESH ps1 buffers (bufs=4) -> each boundary matmul carries only the DMA
# wait, never DMA wait + PSUM-bank-WAR wait (matmul has ONE wait slot).
# Total HWDGE DMAs = wk1 + 3 xp + 2 shifts + 2 outs = 8 = #physical queues,
# so NO queue is reused and NO DMA ever carries a queue-WAW wait. First xp
# piece is tiny so stage-1 chunk 0 starts early; boundaries at chunks 0,1,3
# all use fresh ps1 buffers (bufs=4).
XP_PC = [(0, 9), (9, 18), (27, 47)]
BD_CH = [(40, 8), (48, 8)]  # block-diag stage-2 w chunks
S2_CH = [(0, 8), (8, 8), (16, 8), (24, 8), (32, 8)]  # shifted stage-2 w chunks
# out DMA pieces (w ranges), in issue order; last computed piece is small
# wk free cols: w1 (5*112) + w2 (128) + 7 zero-padded block-diag w2 blocks
W2OFF = KH * M1  # 560
BDOFF = W2OFF + 128  # 688
WKC = BDOFF + KW * 128  # 1584
BF16 = mybir.dt.bfloat16
F32 = mybir.dt.float32

_NC = None


def _build_nc(attempt=0):
    nc = bass.Bass(
        "TRN2",
        target_bir_lowering=False,
        debug=False,
        enable_asserts=False,
        num_devices=N,
    )
    xp_d = nc.dram_tensor("xp", [C, WP, RP], BF16, kind="ExternalInput")
    wk_d = nc.dram_tensor("wk", [C, WKC], BF16, kind="ExternalInput")
    out_d = nc.dram_tensor("out", [C, W * H], BF16, kind="ExternalOutput")

    with tile.TileContext(nc) as tc:
        # schedule perturbation for compile-retry (Tile scheduler flake)
        for _ in range(attempt):
            nc.sync.nop(nofuse=True)
        with (
            tc.tile_pool(name="const", bufs=1) as constp,
            tc.tile_pool(name="xin", bufs=1) as xinp,
            tc.tile_pool(name="p2s", bufs=1) as p2sp,
            tc.tile_pool(name="p2a", bufs=1) as p2ap,
            tc.tile_pool(name="outs", bufs=1) as outsp,
            tc.tile_pool(name="warm", bufs=1) as warmp,
            tc.tile_pool(name="psd", bufs=1, space="PSUM") as psd,
            tc.tile_pool(name="ps1", bufs=4, space="PSUM") as ps1,
            tc.tile_pool(name="ps2", bufs=3, space="PSUM") as ps2,
        ):
            in_dmas = []
            wk_t = constp.tile([C, WKC], BF16, tag="wk")
            wk_ap = wk_d.ap()
            xp_t = xinp.tile([C, WP, RP], BF16, tag="xp")
            xp_ap = xp_d.ap()
            # xp piece 0 issued FIRST (it gates stage-1 chunk 0); then the
            # critical weights (w1 + w2) on a small fast HWDGE DMA; the
            # block-diag blocks (needed only mid-kernel) via SWDGE so the
            # 8 HWDGE queues stay exclusive (no queue-reuse WAW waits).
            w0, wc = XP_PC[0]
            in_dmas.append(
                nc.sync.dma_start(xp_t[:, w0 : w0 + wc, :], xp_ap[:, w0 : w0 + wc, :])
            )
            in_dmas.append(nc.sync.dma_start(wk_t[:, 0:BDOFF], wk_ap[:, 0:BDOFF]))
            wkbd_dma = nc.gpsimd.dma_start(wk_t[:, BDOFF:], wk_ap[:, BDOFF:])
            w1_t = wk_t[:, 0:W2OFF].rearrange("c (kh m) -> c kh m", kh=KH)
            w2_t = wk_t[0:M1, W2OFF:BDOFF]  # [112, 128] = tile(w45.T, (7,1))
            # block-diag stage-2 weights: wbd[g] zero except rows 16g:16g+16
            wbd_t = wk_t[0:M1, BDOFF:].rearrange("p (g o) -> p g o", g=KW)

            for w0, wc in XP_PC[1:]:
                in_dmas.append(
                    nc.sync.dma_start(
                        xp_t[:, w0 : w0 + wc, :], xp_ap[:, w0 : w0 + wc, :]
                    )
                )


            p2s_t = p2sp.tile([M1, WP, H], BF16)
            p2a_t = p2ap.tile([M1, WSPLIT, H], BF16)
            outsb_t = outsp.tile([C, W, H], BF16)
            out_ap = out_d.ap()

            # PE pstate warmup: the PE reaches full clock only ~6us after
            # its first activity, and idle gaps reset the ramp. Bridge from
            # kernel start into stage-1 with a gapless warmup spree on a
            # memset tile (DVE memset is DVE's first instruction, ~0.6us).
            dt = psd.tile([1, 504], F32, tag="dummy")
            warm_t = warmp.tile([C, 504], BF16, tag="warm")
            nc.vector.memset(warm_t[:], 0.0)
            for _ in range(11):
                nc.tensor.matmul(
                    dt[:], warm_t[:, 0:1], warm_t[:], start=True, stop=True
                )
            # dummy matmul absorbing the wk-DMA queue tick (PE single-wait)
            wk_dummy = nc.tensor.matmul(
                dt[:, 0:1], wk_t[0:M1, 0:1], wk_t[0:M1, 0:1], start=True, stop=True
            )


            # ---- stage 1: 9 chunks x 5 kh taps ----
            # all stage-1 evacs on ONE engine (DVE) so each shift DMA's wait
            # collapses to a single monotonic semaphore value.
            last_s1_mm = None
            for k, (w0, wc) in enumerate(S1_CH):
                pt = ps1.tile([M1, wc, H], F32, tag="p1")
                for kh in range(KH):
                    last_s1_mm = nc.tensor.matmul(
                        pt[:],
                        w1_t[:, kh, :],
                        xp_t[:, w0 : w0 + wc, DH * kh : DH * kh + H],
                        start=(kh == 0),
                        stop=(kh == KH - 1),
                    )
                nc.vector.tensor_copy(p2s_t[:, w0 : w0 + wc, :], pt[:])

            # absorb in-DMA completion ticks into SP program order BEFORE the
            # shift DMAs: the 8 physical HWDGE queues round-robin, so shifts
            # reuse in-DMA queues; covering those ticks here removes the WAW
            # queue wait from the single-wait-slot shift DMAs.
            for d in in_dmas:
                nop = nc.sync.nop(nofuse=True)
                add_dep_helper(nop.ins, d.ins, sync=True, reason="absorb in tick")

            # ---- shift DMAs (issue as soon as their p2s chunks land) ----
            # g=0..4 on the Pool SWDGE queue (ONE completion semaphore for all
            # five; ~1us software descriptor-gen each on the otherwise-idle
            # Pool engine, started early and overlapped with stage 1). g=5,6
            # (gated on the last-needed stage-1 chunk) on the SP HWDGE path.
            shift_sw = []
            for g in range(5):
                shift_sw.append(
                    nc.gpsimd.dma_start(
                        p2a_t[16 * g : 16 * g + 16, :, :],
                        p2s_t[16 * g : 16 * g + 16, 3 * g : 3 * g + WSPLIT, :],
                    )
                )
            shift_hw = []
            for g in (5, 6):
                shift_hw.append(
                    nc.sync.dma_start(
                        p2a_t[16 * g : 16 * g + 16, :, :],
                        p2s_t[16 * g : 16 * g + 16, 3 * g : 3 * g + WSPLIT, :],
                    )
                )
            shift_dmas = shift_sw + shift_hw
            out_dmas = []
            last_mm = None
            last_cp = None

            def out_dma(dram_slice, sbuf_slice, evac):
                # SP HWDGE, fresh queue (no WAW): carries only the Act-evac
                # semaphore wait.
                out_dmas.append(nc.sync.dma_start(dram_slice, sbuf_slice))

            # PE dummy absorbing the SWDGE wkbd-weights tick before the
            # block-diag matmuls need them (1 wait; met long before BD).
            bd_dummy = nc.tensor.matmul(
                dt[:, 0:1], wk_t[0:M1, 0:1], wk_t[0:M1, 0:1], start=True, stop=True
            )
            add_dep_helper(
                bd_dummy.ins, last_s1_mm.ins, sync=True, reason="after stage 1"
            )
            add_dep_helper(
                bd_dummy.ins, wkbd_dma.ins, sync=True, reason="absorb wkbd tick"
            )

            # ---- stage 2a: block-diag tail (w >= WSPLIT), no DMA dep ----
            for j, (w0, wc) in enumerate(BD_CH):
                qt = ps2.tile([C, wc, H], F32, tag="p2")
                for g in range(KW):
                    last_mm = nc.tensor.matmul(
                        qt[:],
                        wbd_t[:, g, :],
                        p2s_t[:, w0 + 3 * g : w0 + 3 * g + wc, :],
                        start=(g == 0),
                        stop=(g == KW - 1),
                    )
                last_cp = nc.scalar.copy(outsb_t[:, w0 : w0 + wc, :], qt[:])
            out_dma(out_ap[:, WSPLIT * H :], outsb_t[:, WSPLIT:, :], last_cp)

            # ---- PE dummies absorbing the 7 shift-DMA completion ticks ----
            # One 1-col matmul per shift DMA (matmul = 1 wait slot; each
            # SWDGE DMA gets its OWN DMASW sem, so all 7 need absorbing),
            # ordered after the block-diag tail on PE, so stage-2b matmuls'
            # p2a deps are covered by PE program order.
            prev_mm = last_mm
            for d in shift_dmas:
                dmm = nc.tensor.matmul(
                    dt[:, 0:1], wk_t[0:M1, 0:1], wk_t[0:M1, 0:1], start=True, stop=True
                )
                add_dep_helper(dmm.ins, prev_mm.ins, sync=True, reason="after BD")
                add_dep_helper(dmm.ins, d.ins, sync=True, reason="absorb shift tick")
                prev_mm = dmm

            # ---- stage 2b: shifted single-matmul chunks (w < WSPLIT) ----
            for j, (w0, wc) in enumerate(S2_CH):
                qt = ps2.tile([C, wc, H], F32, tag="p2")
                last_mm = nc.tensor.matmul(
                    qt[:], w2_t[:], p2a_t[:, w0 : w0 + wc, :], start=True, stop=True
                )
                add_dep_helper(
                    last_mm.ins, prev_mm.ins, sync=True, reason="after shift dummies"
                )
                prev_mm = last_mm
                # s2b evacs on DVE (idle after stage 1) so PE isn't throttled
                # by Act evac latency via PSUM-bank WAR, and Act is free.
                last_cp = nc.vector.tensor_copy(outsb_t[:, w0 : w0 + wc, :], qt[:])
                if j == 3:  # w 0..31 done
                    out_dma(out_ap[:, 0 : 32 * H], outsb_t[:, 0:32, :], last_cp)
                elif j == 4:  # w 32..39 done: small last piece via SWDGE
                    out_dmas.append(
                        nc.gpsimd.dma_start(
                            out_ap[:, 32 * H : WSPLIT * H], outsb_t[:, 32:WSPLIT, :]
                        )
                    )

            # absorb DMA/engine completion ticks into SP program order so the
            # kernel-tail Drain needs no (or one) semaphore wait per proc.
            # (in_dmas already absorbed above.)
            for dep in shift_dmas + out_dmas + [wkbd_dma, last_mm, last_cp]:
                nop = nc.sync.nop(nofuse=True)
                add_dep_helper(nop.ins, dep.ins, sync=True, reason="absorb tick")
    return nc


def _get_nc():
    global _NC
    if _NC is None:
        _NC = _build_nc()
    return _NC


def _prep_inputs(x, w3, w4, w5):
    w45 = (w5.astype(np.float64) @ w4.astype(np.float64)).astype(np.float32)
    # w1[c, kh, kw*CO+co] = w3[co, c, kh, kw]
    w1 = np.transpose(w3, (1, 2, 3, 0)).reshape(C, KH * M1)
    # w2[kw*CO+co, o] = w45[o, co]
    w2 = np.tile(w45.T, (KW, 1))  # [112, 128]
    wk = np.zeros((C, WKC), np.float32)
    wk[:, :W2OFF] = w1
    wk[:M1, W2OFF:BDOFF] = w2
    for g in range(KW):
        wk[16 * g : 16 * g + 16, BDOFF + 128 * g : BDOFF + 128 * (g + 1)] = w45.T
    wk = wk.astype(ml_dtypes.bfloat16)
    # xp[c, w', r] = xpad[c, r, w']  (padded, W-major)
    xp = np.zeros((N, C, WP, RP), np.float32)
    xp[:, :, PW : PW + W, PH : PH + H] = np.transpose(x, (0, 1, 3, 2))
    xp = xp.astype(ml_dtypes.bfloat16)
    return xp, wk


def kernel(x, w3, w4, w5, trace=False):
    x = np.asarray(x, np.float32)
    w3 = np.asarray(w3, np.float32)
    w4 = np.asarray(w4, np.float32)
    w5 = np.asarray(w5, np.float32)
    xp, wk = _prep_inputs(x, w3, w4, w5)
    in_maps = [{"xp": np.ascontiguousarray(xp[n]), "wk": wk} for n in range(N)]
    global _NC
    res = None
    last_err = None
    for attempt in range(6):
        if _NC is None:
            _NC = _build_nc(attempt)
        try:
            res = run_bass_kernel_spmd(
                _NC, in_maps, core_ids=list(range(N)), trace=trace
            )
            break
        except Exception as e:  # compile-schedule flake: rebuild perturbed
            last_err = e
            _NC = None
    if res is None:
        raise last_err
    # device output is [C, w, h] bf16 -> [C, h, w] f32
    out = np.stack(
        [
            np.transpose(
                np.asarray(res.results[n]["out"]).astype(np.float32).reshape(C, W, H),
                (0, 2, 1),
            )
            for n in range(N)
        ]
    )
    if trace:
        return out, res
    return out
